# revision 2
# baseline (speedup 1.0000x reference)
"""Trainium2 Bass kernel for nn_CrossAttention_47004122087816.

Math (faithful to the reference's "buggy einsum"):
    xn   = LayerNorm(x) * ln_w + ln_b
    q    = (xn @ Wq) * SCALE            [n, E]
    k, v = split(media @ Wkv)           [m, E] each
    sim  = q @ k^T                      [n, m]
    colsum[j] = sum_i softmax(sim, -1)[i, j]
    out  = (colsum[:, None] * v) @ Wout [m, D]

Key observation: attn @ v is never needed — only the column sums of the
softmax.  colsum[j] = sum_i exp(sim[i,j]) / Z_i, so per 128-row tile of sim
we exp (ScalarE, with free running row-sum via accum_out), compute c = 1/Z,
and accumulate colsum via a [128,1]^T @ [128,512] matmul into PSUM.

Sharding: pure data-parallel — batch b=8 over 8 NeuronCores, one batch
element per core, no collectives.

All matmuls run in bf16 (PE array full rate); softmax skips max-subtraction
(sim values are bounded by ~±15 here, exp fits fp32/bf16 comfortably).
Activations are transposed via the DMA xbar (bf16 bounce through DRAM
scratch), keeping the TensorEngine free of transpose work.
"""

import sys

for _p in ("/opt/trn_rl_repo",):
    if _p not in sys.path:
        sys.path.insert(0, _p)

import numpy as np

import concourse.bass as bass  # noqa: F401  (import registers engine types)
import concourse.tile as tile
from concourse import bacc, mybir
from concourse.bass_utils import run_bass_kernel_spmd

B = 8
N = 2048          # x rows per batch element
M = 2048          # media rows per batch element
D = 1024          # model dim
E = 512           # inner dim
P = 128           # partitions
F = 512           # matmul free-dim chunk (one PSUM bank of fp32)
CT = D // P       # 8  c-tiles (contraction over model dim)
ET = E // P       # 4  e-tiles (contraction over inner dim)
NT = N // P       # 16 row tiles
JC = M // F       # 4  column chunks of 512
SCALE = 64 ** -0.5
EPS = 1e-5

FP = mybir.dt.float32
BF = mybir.dt.bfloat16

AF = mybir.ActivationFunctionType
ALU = mybir.AluOpType
AX = mybir.AxisListType


def _build():
    nc = bacc.Bacc("TRN2", target_bir_lowering=False, debug=False, num_devices=B)

    x = nc.dram_tensor("x", [N, D], FP, kind="ExternalInput").ap()
    media = nc.dram_tensor("media", [M, D], FP, kind="ExternalInput").ap()
    ln_w = nc.dram_tensor("ln_w", [D], FP, kind="ExternalInput").ap()
    ln_b = nc.dram_tensor("ln_b", [D], FP, kind="ExternalInput").ap()
    Wq = nc.dram_tensor("Wq", [D, E], FP, kind="ExternalInput").ap()
    Wkv = nc.dram_tensor("Wkv", [D, 2 * E], FP, kind="ExternalInput").ap()
    Wout = nc.dram_tensor("Wout", [E, D], FP, kind="ExternalInput").ap()
    out = nc.dram_tensor("out", [M, D], FP, kind="ExternalOutput").ap()

    with tile.TileContext(nc) as tc:
        from contextlib import ExitStack

        with ExitStack() as ctx:
            consts = ctx.enter_context(tc.tile_pool(name="consts", bufs=1))
            acts = ctx.enter_context(tc.tile_pool(name="acts", bufs=1))
            xstage = ctx.enter_context(tc.tile_pool(name="xstage", bufs=3))
            expp = ctx.enter_context(tc.tile_pool(name="expp", bufs=2))
            small = ctx.enter_context(tc.tile_pool(name="small", bufs=4))
            outst = ctx.enter_context(tc.tile_pool(name="outst", bufs=3))
            psum_mm = ctx.enter_context(
                tc.tile_pool(name="psum_mm", bufs=3, space="PSUM")
            )
            psum_cs = ctx.enter_context(
                tc.tile_pool(name="psum_cs", bufs=4, space="PSUM")
            )
            dram = ctx.enter_context(tc.tile_pool(name="dram", bufs=1, space="DRAM"))

            # ---------------- phase W: weights ----------------
            # ln params striped [P, CT]: column t holds elements t*128..t*128+127
            lnw = consts.tile([P, CT], FP)
            lnb_f = consts.tile([P, CT], FP)
            for t in range(CT):
                nc.sync.dma_start(lnw[:, t : t + 1], ln_w[t * P : (t + 1) * P])
                nc.sync.dma_start(lnb_f[:, t : t + 1], ln_b[t * P : (t + 1) * P])
            lnw_s = consts.tile([P, CT], FP)
            nc.vector.tensor_scalar_mul(lnw_s[:], lnw[:], SCALE)
            lnb_s = consts.tile([P, CT], BF)  # ln_b * SCALE, lhsT for q0
            nc.vector.tensor_scalar_mul(lnb_s[:], lnb_f[:], SCALE)

            # weights: cast f32->bf16 during the DMA itself (SWDGE)
            wq_b = consts.tile([P, CT, E], BF)
            nc.gpsimd.dma_start(wq_b[:], Wq.rearrange("(kt p) d -> p kt d", p=P))
            wkv_b = consts.tile([P, CT, 2 * E], BF)
            nc.gpsimd.dma_start(wkv_b[:], Wkv.rearrange("(kt p) e -> p kt e", p=P))
            wout_b = consts.tile([P, ET, D], BF)
            nc.gpsimd.dma_start(wout_b[:], Wout.rearrange("(et p) d -> p et d", p=P))

            # q0 = (SCALE * ln_b) @ Wq  -> row bias for q
            q0_ps = psum_cs.tile([1, E], FP, tag="cs")
            for kt in range(CT):
                nc.tensor.matmul(
                    q0_ps[:],
                    lhsT=lnb_s[:, kt : kt + 1],
                    rhs=wq_b[:, kt, :],
                    start=(kt == 0),
                    stop=(kt == CT - 1),
                )
            q0_sb = consts.tile([1, E], FP)
            nc.scalar.copy(q0_sb[:], q0_ps[:])
            q0T = consts.tile([P, ET], FP)  # q0 transposed into partition layout
            for t in range(ET):
                nc.sync.dma_start(q0T[:, t : t + 1], q0_sb[0:1, t * P : (t + 1) * P])

            # Wq' = (SCALE * ln_w) ⊙_rows Wq   (bf16)
            wq_p = consts.tile([P, CT, E], BF)
            for kt in range(CT):
                nc.vector.tensor_scalar_mul(
                    wq_p[:, kt], wq_b[:, kt], lnw_s[:, kt : kt + 1]
                )

            eps_t = consts.tile([P, 1], FP)
            nc.vector.memset(eps_t[:], EPS)

            # ---------------- phase X1: LayerNorm x -> xhat (bf16, via DRAM) ----
            xhat_dram = dram.tile([N, D], BF)
            for rt in range(NT):
                xt = xstage.tile([P, D], FP, tag="xt")
                nc.sync.dma_start(xt[:], x[rt * P : (rt + 1) * P, :])
                st = small.tile([P, 2, 6], FP, tag="st")
                for sg in range(2):
                    nc.vector.bn_stats(st[:, sg, :], xt[:, sg * 512 : (sg + 1) * 512])
                mv = small.tile([P, 2], FP, tag="mv")
                nc.vector.bn_aggr(mv[:], st[:])
                sd = small.tile([P, 1], FP, tag="sd")
                nc.scalar.activation(
                    sd[:], mv[:, 1:2], func=AF.Sqrt, bias=eps_t[:], scale=1.0
                )
                rsig = small.tile([P, 1], FP, tag="rsig")
                nc.vector.reciprocal(rsig[:], sd[:])
                nmr = small.tile([P, 1], FP, tag="nmr")  # -mu * rsig
                nc.vector.tensor_scalar(
                    nmr[:], mv[:, 0:1], rsig[:], -1.0, ALU.mult, ALU.mult
                )
                xh = xstage.tile([P, D], BF, tag="xh")
                nc.scalar.activation(
                    xh[:], xt[:], func=AF.Identity, bias=nmr[:], scale=rsig[:]
                )
                nc.sync.dma_start(xhat_dram[rt * P : (rt + 1) * P, :], xh[:])

            # ---------------- phase M: media -> kT, vT ----------------
            media_bf = dram.tile([M, D], BF)
            for rc in range(4):
                nc.gpsimd.dma_start(
                    media_bf[rc * 512 : (rc + 1) * 512, :],
                    media[rc * 512 : (rc + 1) * 512, :],
                )
            mediaT = acts.tile([P, CT, M], BF)
            for ct in range(CT):
                nc.sync.dma_start_transpose(
                    mediaT[:, ct, :], media_bf[:, ct * P : (ct + 1) * P]
                )
            kT = acts.tile([P, ET, M], BF)
            vT = acts.tile([P, ET, M], BF)
            for jc in range(JC):
                for et in range(2 * ET):
                    ps = psum_mm.tile([P, F], FP, tag="ps")
                    for kt in range(CT):
                        nc.tensor.matmul(
                            ps[:],
                            lhsT=wkv_b[:, kt, et * P : (et + 1) * P],
                            rhs=mediaT[:, kt, jc * F : (jc + 1) * F],
                            start=(kt == 0),
                            stop=(kt == CT - 1),
                        )
                    if et < ET:
                        dst = kT[:, et, jc * F : (jc + 1) * F]
                    else:
                        dst = vT[:, et - ET, jc * F : (jc + 1) * F]
                    if et % 2 == 0:
                        nc.scalar.copy(dst, ps[:])
                    else:
                        nc.vector.tensor_copy(dst, ps[:])

            # ---------------- phase X2: xhatT, qT ----------------
            xhatT = acts.tile([P, CT, N], BF)
            for ct in range(CT):
                nc.sync.dma_start_transpose(
                    xhatT[:, ct, :], xhat_dram[:, ct * P : (ct + 1) * P]
                )
            qT = acts.tile([P, ET, N], BF)
            for ic in range(JC):
                for dt in range(ET):
                    ps = psum_mm.tile([P, F], FP, tag="ps")
                    for kt in range(CT):
                        nc.tensor.matmul(
                            ps[:],
                            lhsT=wq_p[:, kt, dt * P : (dt + 1) * P],
                            rhs=xhatT[:, kt, ic * F : (ic + 1) * F],
                            start=(kt == 0),
                            stop=(kt == CT - 1),
                        )
                    # evac with q0 row-bias folded in
                    nc.scalar.activation(
                        qT[:, dt, ic * F : (ic + 1) * F],
                        ps[:],
                        func=AF.Identity,
                        bias=q0T[:, dt : dt + 1],
                        scale=1.0,
                    )

            # ---------------- phase S: sim, exp, colsum ----------------
            csum = [
                psum_cs.tile([1, F], FP, tag="cs", name=f"cs{i}") for i in range(JC)
            ]
            exs: list = [None, None]  # software pipeline: colsum lags sim by 1
            zrbs: list = [None, None]

            def colsum_mms(it):
                ex_p, zrb_p = exs[it % 2], zrbs[it % 2]
                for jc in range(JC):
                    nc.tensor.matmul(
                        csum[jc][:],
                        lhsT=zrb_p[:],
                        rhs=ex_p[:, jc * F : (jc + 1) * F],
                        start=(it == 0),
                        stop=(it == NT - 1),
                        skip_group_check=True,
                    )

            for it in range(NT):
                ex = expp.tile([P, M], BF, tag="ex")
                zp = small.tile([P, JC], FP, tag="zp")
                for jc in range(JC):
                    ps = psum_mm.tile([P, F], FP, tag="ps")
                    for et in range(ET):
                        nc.tensor.matmul(
                            ps[:],
                            lhsT=qT[:, et, it * P : (it + 1) * P],
                            rhs=kT[:, et, jc * F : (jc + 1) * F],
                            start=(et == 0),
                            stop=(et == ET - 1),
                        )
                    nc.scalar.activation(
                        ex[:, jc * F : (jc + 1) * F],
                        ps[:],
                        func=AF.Exp,
                        accum_out=zp[:, jc : jc + 1],
                    )
                z = small.tile([P, 1], FP, tag="z")
                nc.vector.tensor_reduce(z[:], zp[:], axis=AX.X, op=ALU.add)
                zr = small.tile([P, 1], FP, tag="zr")
                nc.vector.reciprocal(zr[:], z[:])
                zrb = small.tile([P, 1], BF, tag="zrb")
                nc.vector.tensor_copy(zrb[:], zr[:])
                exs[it % 2], zrbs[it % 2] = ex, zrb
                if it > 0:
                    colsum_mms(it - 1)
            colsum_mms(NT - 1)

            csum_sb = consts.tile([1, M], FP)
            for jc in range(JC):
                nc.scalar.copy(csum_sb[0:1, jc * F : (jc + 1) * F], csum[jc][:])
            scol = consts.tile([P, NT], FP)  # colsum in partition layout
            for t in range(NT):
                nc.sync.dma_start(
                    scol[:, t : t + 1], csum_sb[0:1, t * P : (t + 1) * P]
                )

            # ---------------- phase F: out = (colsum ⊙ v) @ Wout ----------------
            for jt in range(NT):
                ot = outst.tile([P, D], FP, tag="ot")
                for n2 in range(2):
                    ps = psum_mm.tile([P, F], FP, tag="ps")
                    for et in range(ET):
                        nc.tensor.matmul(
                            ps[:],
                            lhsT=vT[:, et, jt * P : (jt + 1) * P],
                            rhs=wout_b[:, et, n2 * F : (n2 + 1) * F],
                            start=(et == 0),
                            stop=(et == ET - 1),
                        )
                    if n2 == 0:
                        nc.scalar.mul(
                            ot[:, n2 * F : (n2 + 1) * F], ps[:], scol[:, jt : jt + 1]
                        )
                    else:
                        nc.vector.tensor_scalar_mul(
                            ot[:, n2 * F : (n2 + 1) * F], ps[:], scol[:, jt : jt + 1]
                        )
                nc.sync.dma_start(out[jt * P : (jt + 1) * P, :], ot[:])

    nc.compile()
    return nc


_NC_CACHE = None


def _get_nc():
    global _NC_CACHE
    if _NC_CACHE is None:
        _NC_CACHE = _build()
    return _NC_CACHE


def _run(inputs, trace=False, **kw):
    nc = _get_nc()
    shared = {
        k: np.ascontiguousarray(np.asarray(inputs[k], dtype=np.float32))
        for k in ("ln_w", "ln_b", "Wq", "Wkv", "Wout")
    }
    xs = np.ascontiguousarray(np.asarray(inputs["x"], dtype=np.float32))
    ms = np.ascontiguousarray(np.asarray(inputs["media"], dtype=np.float32))
    in_maps = [dict(shared, x=xs[b], media=ms[b]) for b in range(B)]
    res = run_bass_kernel_spmd(nc, in_maps, core_ids=list(range(B)), trace=trace, **kw)
    out = np.stack([res.results[b]["out"] for b in range(B)], axis=0)
    return out, res


def kernel(**inputs) -> np.ndarray:
    out, _ = _run(inputs, trace=False)
    return out
